# revision 1
# baseline (speedup 1.0000x reference)
"""Trainium2 Bass kernel for the DEER-MLP spiking network.

Network: x(4,32,196,384) -> FC1(384->1536) -> BatchNorm -> LIF(T=4) ->
FC2(1536->384) -> BatchNorm -> LIF -> spikes(4,32,196,384).

Math note: the reference solves the LIF recurrence with 10 DEER Newton
iterations over T=4 steps. Newton on a length-T triangular system is exact
after T iterations, so the converged result equals the plain sequential
recurrence; we compute that directly (4 elementwise steps).

Distribution: data-parallel over the flattened B*N batch across 8 cores
(784 lanes/core). BatchNorm statistics are the only cross-core coupling:
two tiny AllReduces ([128,24] and [128,6] fp32).

Precision: both matmuls run as multi-pass fp16 with operands split into
hi/lo fp16 limbs (split on host; the PE honors fp16 subnormals, verified
on hardware). fp16 products accumulate exactly into fp32 PSUM, so
FC1 = x_hi@w_hi + x_lo@w_hi + x_hi@w_lo reproduces fp32 to ~2^-22 (the
dropped lo@lo term), and FC2's spikes are exactly 0/1 in fp16 so two
passes (w_hi + w_lo) are ~2^-22 as well. This is 4x (FC2) / 1.33x (FC1)
faster than native fp32 matmul on the PE at fp32-level accuracy.

Per-core pipeline (single NEFF):
  A: FC1 on PE; bias add + per-channel sum/sumsq fused into the PSUM
     evacuation on the Scalar engine (accum_out); y1 -> DRAM scratch.
     AllReduce BN1 stats.
  B: BN1 affine + 4-step LIF on DVE; spikes stored fp16; FC2 fp16;
     BN2 stats fused in evacuation; y2 kept SBUF-resident. AllReduce.
  C: BN2 affine + LIF on DVE in place.
  D: PE-transpose spikes back to row-major, DMA out.

Host-side prep in kernel(): shard x over B, pre-transpose to [C, R] and
split into fp16 limbs; pre-transpose W1/W2 and split into fp16 limbs.
"""

import numpy as np

import concourse.bass as bass
import concourse.mybir as mybir
import concourse.tile as tile
from concourse import bacc
from concourse.bass_utils import run_bass_kernel_spmd
F32 = mybir.dt.float32
F16 = mybir.dt.float16
AF = mybir.ActivationFunctionType
OP = mybir.AluOpType
AX = mybir.AxisListType

T, B, NN, C, H = 4, 32, 196, 384, 1536
NCORES = 8
BLOC = B // NCORES            # 4 batches per core
MLOC = BLOC * NN              # 784 lanes per core
R = T * MLOC                  # 3136 flattened (t, m) rows per core
NTOT = T * B * NN             # 25088 batchnorm samples per channel
KC = C // 128                 # 3 c-tiles
KH = H // 128                 # 12 h-tiles
EPS = 1e-5
P = 128

A_CHUNKS = [(i * 512, 512) for i in range(R // 512)] + [(R - R % 512, R % 512)]
B_CHUNKS = [(0, MLOC // 2), (MLOC // 2, MLOC // 2)]


def _lif(nc, pool, drive, s_out, mlen, tag):
    """Sequential LIF over T steps.

    drive: [128, T, >=mlen] fp32 (already 0.5*BN(y)); s_out: [128, T, >=mlen]
    spike output. h_t = 0.5*v_{t-1} + drive_t; s = (h>=1); v = h*(h<1).
    s_out may alias drive (phase C writes spikes in place).
    """
    v = pool.tile([P, mlen], F32, tag=f"{tag}_v", name=f"{tag}_v")
    nc.vector.scalar_tensor_tensor(
        v[:], drive[:, 0, :mlen], 1.0, drive[:, 0, :mlen], OP.is_lt, OP.mult
    )
    nc.vector.tensor_scalar(
        s_out[:, 0, :mlen], drive[:, 0, :mlen], 1.0, None, OP.is_ge
    )
    for t in range(1, T):
        h = pool.tile([P, mlen], F32, tag=f"{tag}_h", name=f"{tag}_h")
        nc.vector.scalar_tensor_tensor(
            h[:], v[:], 0.5, drive[:, t, :mlen], OP.mult, OP.add
        )
        if t < T - 1:
            v = pool.tile([P, mlen], F32, tag=f"{tag}_v", name=f"{tag}_v")
            nc.vector.scalar_tensor_tensor(v[:], h[:], 1.0, h[:], OP.is_lt, OP.mult)
        nc.vector.tensor_scalar(s_out[:, t, :mlen], h[:], 1.0, None, OP.is_ge)


def _bn_coeffs(nc, pool, stg, gt, bet, k, tag):
    """From allreduced [128, 2k] (sum || sumsq) compute the fused affine
    drive = y*dsc + dsh  ==  0.5 * ((y - mean) * rsqrt(var+eps) * g + be)."""
    mean = pool.tile([P, k], F32, tag=f"{tag}_mean", name=f"{tag}_mean")
    nc.vector.tensor_scalar(mean[:], stg[:, 0:k], 1.0 / NTOT, None, OP.mult)
    var = pool.tile([P, k], F32, tag=f"{tag}_var", name=f"{tag}_var")
    nc.vector.tensor_scalar(var[:], stg[:, k : 2 * k], 1.0 / NTOT, None, OP.mult)
    msq = pool.tile([P, k], F32, tag=f"{tag}_msq", name=f"{tag}_msq")
    nc.vector.tensor_tensor(msq[:], mean[:], mean[:], OP.mult)
    nc.vector.tensor_tensor(var[:], var[:], msq[:], OP.subtract)
    nc.vector.tensor_scalar(var[:], var[:], EPS, None, OP.add)
    std = pool.tile([P, k], F32, tag=f"{tag}_std", name=f"{tag}_std")
    nc.scalar.activation(std[:], var[:], AF.Sqrt, bias=0.0, scale=1.0)
    rstd = pool.tile([P, k], F32, tag=f"{tag}_rstd", name=f"{tag}_rstd")
    nc.vector.reciprocal(rstd[:], std[:])
    dsc = pool.tile([P, k], F32, tag=f"{tag}_dsc", name=f"{tag}_dsc")
    nc.vector.tensor_tensor(dsc[:], rstd[:], gt[:], OP.mult)
    dsh = pool.tile([P, k], F32, tag=f"{tag}_dsh", name=f"{tag}_dsh")
    nc.vector.tensor_tensor(dsh[:], mean[:], dsc[:], OP.mult)
    nc.vector.tensor_tensor(dsh[:], bet[:], dsh[:], OP.subtract)
    nc.vector.tensor_scalar(dsc[:], dsc[:], 0.5, None, OP.mult)
    nc.vector.tensor_scalar(dsh[:], dsh[:], 0.5, None, OP.mult)
    return dsc, dsh


def _build():
    nc = bacc.Bacc("TRN2", target_bir_lowering=False, debug=False,
                   num_devices=NCORES)

    xh_d = nc.dram_tensor("xthi", [KC, P, R], F16, kind="ExternalInput")
    xl_d = nc.dram_tensor("xtlo", [KC, P, R], F16, kind="ExternalInput")
    w1h_d = nc.dram_tensor("w1thi", [KC, P, H], F16, kind="ExternalInput")
    w1l_d = nc.dram_tensor("w1tlo", [KC, P, H], F16, kind="ExternalInput")
    w2h_d = nc.dram_tensor("w2thi", [KH, P, C], F16, kind="ExternalInput")
    w2l_d = nc.dram_tensor("w2tlo", [KH, P, C], F16, kind="ExternalInput")
    b1_d = nc.dram_tensor("b1", [H], F32, kind="ExternalInput")
    g1_d = nc.dram_tensor("g1", [H], F32, kind="ExternalInput")
    be1_d = nc.dram_tensor("be1", [H], F32, kind="ExternalInput")
    b2_d = nc.dram_tensor("b2", [C], F32, kind="ExternalInput")
    g2_d = nc.dram_tensor("g2", [C], F32, kind="ExternalInput")
    be2_d = nc.dram_tensor("be2", [C], F32, kind="ExternalInput")
    out_d = nc.dram_tensor("out", [R, C], F32, kind="ExternalOutput")

    groups = [list(range(NCORES))]

    with tile.TileContext(nc) as tc:
        with (
            tc.tile_pool(name="const", bufs=1) as const,
            tc.tile_pool(name="dram", bufs=1, space="DRAM") as dram,
        ):
            def colvec(dst_k, src):
                t_ = const.tile([P, dst_k], F32, name=f"cv_{src.name}",
                                tag=f"cv_{src.name}")
                nc.sync.dma_start(
                    t_[:], src.ap().rearrange("(a p) -> p a", p=P)
                )
                return t_

            b1t, g1t, be1t = (colvec(KH, d) for d in (b1_d, g1_d, be1_d))
            b2t, g2t, be2t = (colvec(KC, d) for d in (b2_d, g2_d, be2_d))

            w2h = const.tile([P, KH, C], F16)
            nc.sync.dma_start(w2h[:], w2h_d.ap().rearrange("k p c -> p k c"))
            w2l = const.tile([P, KH, C], F16)
            nc.sync.dma_start(w2l[:], w2l_d.ap().rearrange("k p c -> p k c"))

            # --- phase A: FC1 (3-pass bf16) + BN1 partial stats ---------
            y1s = dram.tile([KH, P, R], F32)
            asum1 = const.tile([P, KH, len(A_CHUNKS)], F32)
            asq1 = const.tile([P, KH, len(A_CHUNKS)], F32)
            with (
                tc.tile_pool(name="pax", bufs=1) as pax,
                tc.tile_pool(name="pa", bufs=4) as pa,
                tc.tile_pool(name="ps_mm", bufs=6, space="PSUM") as ps_mm,
            ):
                w1h = pax.tile([P, KC, H], F16)
                nc.sync.dma_start(w1h[:], w1h_d.ap().rearrange("k p h -> p k h"))
                w1l = pax.tile([P, KC, H], F16)
                nc.sync.dma_start(w1l[:], w1l_d.ap().rearrange("k p h -> p k h"))
                xh = pax.tile([P, KC, R], F16)
                nc.sync.dma_start(xh[:], xh_d.ap().rearrange("k p r -> p k r"))
                xl = pax.tile([P, KC, R], F16)
                nc.sync.dma_start(xl[:], xl_d.ap().rearrange("k p r -> p k r"))

                for ci, (r0, rlen) in enumerate(A_CHUNKS):
                    for a in range(KH):
                        ps = ps_mm.tile([P, 512], F32, tag="mm")
                        idx = 0
                        for wt, xt in ((w1h, xh), (w1l, xh), (w1h, xl)):
                            for k in range(KC):
                                nc.tensor.matmul(
                                    ps[:, :rlen],
                                    wt[:, k, a * P : (a + 1) * P],
                                    xt[:, k, r0 : r0 + rlen],
                                    start=(idx == 0),
                                    stop=(idx == 8),
                                )
                                idx += 1
                        y1sb = pa.tile([P, 512], F32, tag="y1sb")
                        nc.scalar.activation(
                            y1sb[:, :rlen], ps[:, :rlen], AF.Identity,
                            bias=b1t[:, a : a + 1], scale=1.0,
                            accum_out=asum1[:, a, ci : ci + 1],
                        )
                        sqt = pa.tile([P, 512], F32, tag="sqt")
                        nc.scalar.activation(
                            sqt[:, :rlen], ps[:, :rlen], AF.Square,
                            bias=b1t[:, a : a + 1], scale=1.0,
                            accum_out=asq1[:, a, ci : ci + 1],
                        )
                        nc.sync.dma_start(y1s[a, :, r0 : r0 + rlen],
                                          y1sb[:, :rlen])

            # --- BN1 stat allreduce -------------------------------------
            # Stats DMAs ride the gpsimd queue: a collective-gated load at
            # the head of the sync queue would head-of-line block the
            # phase-B y1 prefetch below.
            st1 = const.tile([P, 2 * KH], F32)
            nc.vector.tensor_reduce(st1[:, 0:KH], asum1[:], AX.X, OP.add)
            nc.vector.tensor_reduce(st1[:, KH : 2 * KH], asq1[:], AX.X, OP.add)
            st1_in = dram.tile([P, 2 * KH], F32)
            st1_out = dram.tile([P, 2 * KH], F32)
            nc.gpsimd.dma_start(st1_in[:], st1[:])
            nc.gpsimd.collective_compute(
                "AllReduce", OP.add, replica_groups=groups,
                ins=[st1_in.opt()], outs=[st1_out.opt()],
            )
            stg1 = const.tile([P, 2 * KH], F32)
            nc.gpsimd.dma_start(stg1[:], st1_out[:])
            dsc1, dsh1 = _bn_coeffs(nc, const, stg1, g1t, be1t, KH, "bn1")

            # --- phase B: BN1 + LIF1 + FC2 (2-pass fp16) + BN2 stats ----
            y2r = [const.tile([P, T, MLOC], F32, tag=f"y2r{ct}",
                              name=f"y2r{ct}")
                   for ct in range(KC)]
            nb2 = len(B_CHUNKS) * T
            asum2 = const.tile([P, KC, nb2], F32)
            asq2 = const.tile([P, KC, nb2], F32)
            with (
                tc.tile_pool(name="pb", bufs=4) as pb,
                tc.tile_pool(name="pb_s1", bufs=13) as pbs1,
                tc.tile_pool(name="ps_mm2", bufs=4, space="PSUM") as ps_mm2,
            ):
                # Prefetch the first half of chunk-0 y1 while the stat
                # allreduce is in flight (the loads depend only on phase-A
                # scratch writes, not on the collective).
                NPRE = 6
                yt_pre = []
                m0p, mlenp = B_CHUNKS[0]
                for a in range(NPRE):
                    yt = pb.tile([P, T, mlenp], F32, tag="yt", bufs=8,
                                 name=f"yt_pre{a}")
                    src = y1s[a].rearrange("p (t m) -> p t m", t=T)
                    nc.sync.dma_start(yt[:], src[:, :, m0p : m0p + mlenp])
                    yt_pre.append(yt)
                for mi, (m0, mlen) in enumerate(B_CHUNKS):
                    s1_tiles = []
                    for a in range(KH):
                        if mi == 0 and a < NPRE:
                            yt = yt_pre[a]
                        else:
                            yt = pb.tile([P, T, mlen], F32, tag="yt", bufs=8,
                                         name=f"yt{mi}_{a}")
                            src = y1s[a].rearrange("p (t m) -> p t m", t=T)
                            nc.sync.dma_start(yt[:], src[:, :, m0 : m0 + mlen])
                        nc.vector.tensor_scalar(
                            yt[:], yt[:], dsc1[:, a : a + 1],
                            dsh1[:, a : a + 1], OP.mult, OP.add,
                        )
                        st_ = pbs1.tile([P, T, mlen], F16, tag="s1")
                        _lif(nc, pb, yt, st_, mlen, "lif1")
                        s1_tiles.append(st_)
                    for t in range(T):
                        for ct in range(KC):
                            ps2 = ps_mm2.tile([P, 512], F32, tag="mm2")
                            idx = 0
                            for k in range(KH):
                                for wsp in (w2h, w2l):
                                    nc.tensor.matmul(
                                        ps2[:, :mlen],
                                        wsp[:, k, ct * P : (ct + 1) * P],
                                        s1_tiles[k][:, t, :mlen],
                                        start=(idx == 0),
                                        stop=(idx == 2 * KH - 1),
                                    )
                                    idx += 1
                            ci2 = mi * T + t
                            nc.scalar.activation(
                                y2r[ct][:, t, m0 : m0 + mlen], ps2[:, :mlen],
                                AF.Identity, bias=b2t[:, ct : ct + 1],
                                scale=1.0,
                                accum_out=asum2[:, ct, ci2 : ci2 + 1],
                            )
                            sqt2 = pb.tile([P, 512], F32, tag="sqt2")
                            nc.scalar.activation(
                                sqt2[:, :mlen], ps2[:, :mlen], AF.Square,
                                bias=b2t[:, ct : ct + 1], scale=1.0,
                                accum_out=asq2[:, ct, ci2 : ci2 + 1],
                            )

            # --- BN2 stat allreduce -------------------------------------
            st2 = const.tile([P, 2 * KC], F32)
            nc.vector.tensor_reduce(st2[:, 0:KC], asum2[:], AX.X, OP.add)
            nc.vector.tensor_reduce(st2[:, KC : 2 * KC], asq2[:], AX.X, OP.add)
            st2_in = dram.tile([P, 2 * KC], F32)
            st2_out = dram.tile([P, 2 * KC], F32)
            nc.gpsimd.dma_start(st2_in[:], st2[:])
            nc.gpsimd.collective_compute(
                "AllReduce", OP.add, replica_groups=groups,
                ins=[st2_in.opt()], outs=[st2_out.opt()],
            )
            stg2 = const.tile([P, 2 * KC], F32)
            nc.gpsimd.dma_start(stg2[:], st2_out[:])
            dsc2, dsh2 = _bn_coeffs(nc, const, stg2, g2t, be2t, KC, "bn2")

            # --- phase C: BN2 (in place) + LIF2 -> fp16 spikes ----------
            RPAD = ((R + P - 1) // P) * P
            with (
                tc.tile_pool(name="ps2", bufs=1) as ps2,
                tc.tile_pool(name="pc", bufs=3) as pc,
                tc.tile_pool(name="pd", bufs=4) as pd,
            ):
                s2t = [ps2.tile([P, RPAD], F16, tag=f"s2t{ct}",
                                name=f"s2t{ct}")
                       for ct in range(KC)]
                for ct in range(KC):
                    if RPAD > R:
                        nc.vector.memset(s2t[ct][:, R:], 0.0)
                    nc.vector.tensor_scalar(
                        y2r[ct][:], y2r[ct][:], dsc2[:, ct : ct + 1],
                        dsh2[:, ct : ct + 1], OP.mult, OP.add,
                    )
                    s2v = s2t[ct][:, :R].rearrange("p (t m) -> p t m", t=T)
                    _lif(nc, pc, y2r[ct], s2v, MLOC, "lif2")

                # --- phase D: DMA-transpose (XBAR, fp16) + upcast -------
                for r0 in range(0, R, P):
                    rlen = min(P, R - r0)
                    ob16 = pd.tile([P, C], F16, tag="ob16")
                    for ct in range(KC):
                        nc.sync.dma_start_transpose(
                            ob16[:, ct * P : (ct + 1) * P],
                            s2t[ct][:, r0 : r0 + P],
                        )
                    ob = pd.tile([P, C], F32, tag="ob")
                    nc.vector.tensor_copy(ob[:rlen], ob16[:rlen])
                    nc.sync.dma_start(out_d[r0 : r0 + rlen, :], ob[:rlen, :])


    nc.compile()
    return nc


_NC = None
TRACE = False          # set by test harness to capture an NTFF profile
LAST_RESULT = None     # BassKernelResults of the most recent run


def _get_nc():
    global _NC
    if _NC is None:
        _NC = _build()
    return _NC


def _split_f16(a):
    hi = a.astype(np.float16)
    lo = (a - hi.astype(np.float32)).astype(np.float16)
    return np.ascontiguousarray(hi), np.ascontiguousarray(lo)


def _in_maps(x, W1, b1, g1, be1, W2, b2, g2, be2):
    x = np.asarray(x, dtype=np.float32)
    w1t = np.asarray(W1, np.float32).T.reshape(KC, P, H)
    w1thi, w1tlo = _split_f16(w1t)
    w2t = np.asarray(W2, np.float32).T.reshape(KH, P, C)
    w2thi, w2tlo = _split_f16(w2t)
    shared = {
        "w1thi": w1thi, "w1tlo": w1tlo,
        "w2thi": w2thi, "w2tlo": w2tlo,
        "b1": np.asarray(b1, np.float32),
        "g1": np.asarray(g1, np.float32),
        "be1": np.asarray(be1, np.float32),
        "b2": np.asarray(b2, np.float32),
        "g2": np.asarray(g2, np.float32),
        "be2": np.asarray(be2, np.float32),
    }
    in_maps = []
    for i in range(NCORES):
        xt = x[:, i * BLOC : (i + 1) * BLOC].reshape(R, C).T.reshape(KC, P, R)
        xthi, xtlo = _split_f16(xt)
        in_maps.append({"xthi": xthi, "xtlo": xtlo, **shared})
    return in_maps


def kernel(x, W1, b1, g1, be1, W2, b2, g2, be2):
    nc = _get_nc()
    in_maps = _in_maps(x, W1, b1, g1, be1, W2, b2, g2, be2)
    res = run_bass_kernel_spmd(nc, in_maps, core_ids=list(range(NCORES)),
                               trace=TRACE)
    global LAST_RESULT
    LAST_RESULT = res
    out = np.concatenate(
        [res.results[i]["out"].reshape(T, BLOC, NN, C) for i in range(NCORES)],
        axis=1,
    )
    return out

